# revision 40
# baseline (speedup 1.0000x reference)
"""Causal self-attention Trainium2 kernel (B=2, T=2048, C=1024, H=16).

Sharding: 8 cores = 2 batches x 4 head-groups (4 heads/core, Megatron-style
column-parallel QKV + row-parallel proj; the row-parallel all-reduce is the
host-side partial sum in `kernel`).

Per-core strategy (bf16 matmul operands, fp32 PSUM accumulation):
  - qT/kT kept transposed [head_dim, T] with 2 heads packed per 128
    partitions; scores are computed transposed (sT[k, q] = k @ qT) with the
    two heads row-packed on the PE array (K=64 each, concurrent sub-arrays).
  - QKV projection work is software-pipelined into the attention loop:
    stage j (qt/kt cols 512j:512j+512, v rows) is emitted inside the
    attention of q-supertile j-1, so the scalar engine's exp stream runs
    concurrently with QKV matmuls instead of after them.
  - softmax in [k, q] layout: exp on ScalarE from PSUM (scores are O(1), no
    max subtraction); causality via restricted column ranges + one
    [128,128] triangular mask multiply per diagonal block.
  - v natural [T, 64] per head with a ones column, so AV emits softmax
    denominators as row 64 of PSUM for free.  Normalization: DVE
    reciprocal_approx_fast on the den row, f32r rank-1 broadcast matmul
    (1 cyc/row), fp16 broadcast tile, one DVE multiply.
  - v bias is folded into b_proj on the host (softmax weights sum to 1).
  - proj consumes yT directly; per-core partial [T, C] written as bf16 and
    summed on the host.
"""

import sys
from collections import deque

for _p in ("/opt/trn_rl_repo",):
    if _p not in sys.path:
        sys.path.insert(0, _p)

import ml_dtypes
import numpy as np

import concourse.bacc as bacc
import concourse.mybir as mybir
import concourse.tile as tile
from concourse.alu_op_type import AluOpType
from concourse.bass_utils import run_bass_kernel_spmd
from concourse.dve_ops import RECIP_APPROX_FAST_CONSTS, RECIPROCAL_APPROX_FAST

F32 = mybir.dt.float32
F32R = mybir.dt.float32r
BF16 = mybir.dt.bfloat16
FP16 = mybir.dt.float16
NPBF = ml_dtypes.bfloat16
EXP = mybir.ActivationFunctionType.Exp

B, T, C = 2, 2048, 1024
H, HD = 16, 64
HPC = 4          # heads per core
NPAIR = 2        # head pairs per core
CL = HPC * HD    # 256 local channels
NCORES = 8
SCALE = 0.125    # 1/sqrt(64), folded into wq/bqs on the host

TT5 = T // 512   # 4  q supertiles / qkv stages
TT1 = T // 128   # 16 t tiles / k blocks
CCH = C // 128   # 8  contraction chunks


def _build_program():
    nc = bacc.Bacc("TRN2", target_bir_lowering=False, debug=False)

    xT_d = nc.dram_tensor("xT", [C, T], BF16, kind="ExternalInput").ap()
    wq_d = nc.dram_tensor("wq", [C, CL], BF16, kind="ExternalInput").ap()
    wk_d = nc.dram_tensor("wk", [C, CL], BF16, kind="ExternalInput").ap()
    wv_d = nc.dram_tensor("wv", [C, CL], BF16, kind="ExternalInput").ap()
    wp_d = nc.dram_tensor("wp", [CL, C], BF16, kind="ExternalInput").ap()
    bqs_d = nc.dram_tensor("bqs", [128, NPAIR], F32, kind="ExternalInput").ap()
    bks_d = nc.dram_tensor("bks", [128, NPAIR], F32, kind="ExternalInput").ap()
    mtri_d = nc.dram_tensor("mtri", [128, 128], BF16, kind="ExternalInput").ap()
    yp_d = nc.dram_tensor("yp", [T, C], BF16, kind="ExternalOutput").ap()

    with tile.TileContext(nc) as tc:
        _attn_kernel(tc, xT_d, wq_d, wk_d, wv_d, wp_d, bqs_d, bks_d, mtri_d,
                     yp_d)
    nc.compile()
    return nc


def _attn_kernel(tc, xT_d, wq_d, wk_d, wv_d, wp_d, bqs_d, bks_d, mtri_d,
                 yp_d, dbg_d=None):
    nc = tc.nc
    mm = nc.tensor.matmul

    with (
        tc.tile_pool(name="const", bufs=1) as cpool,
        tc.tile_pool(name="big", bufs=1) as bigpool,
        tc.tile_pool(name="work", bufs=1) as wkpool,
        tc.tile_pool(name="outp", bufs=1) as opool,
        tc.tile_pool(name="ps", bufs=1, space="PSUM") as ps,
    ):
        # ---- constants ----
        bqs = cpool.tile([128, NPAIR], F32)
        nc.sync.dma_start(bqs, bqs_d)
        bks = cpool.tile([128, NPAIR], F32)
        nc.sync.dma_start(bks, bks_d)
        mtri2 = cpool.tile([128, 2, 128], BF16)
        nc.sync.dma_start(mtri2[:, 0, :], mtri_d)
        nc.sync.dma_start(mtri2[:, 1, :], mtri_d)
        ones64 = cpool.tile([1, HD], BF16)
        nc.vector.memset(ones64, 1.0)
        warm = cpool.tile([1, 512], BF16)
        nc.vector.memset(warm, 1.0)

        # ---- inputs: per-chunk tiles, DMAs issued in priority order ----
        xt = [bigpool.tile([128, T], BF16, name=f"xt{c}") for c in range(CCH)]
        wqt = [bigpool.tile([128, CL], BF16, name=f"wqt{c}") for c in range(CCH)]
        wkt = [bigpool.tile([128, CL], BF16, name=f"wkt{c}") for c in range(CCH)]
        wvt = [bigpool.tile([128, CL], BF16, name=f"wvt{c}") for c in range(CCH)]
        wpt = [bigpool.tile([128, C], BF16, name=f"wpt{p}") for p in range(NPAIR)]

        # stage-0 inputs first; issue from both HWDGE queues (SP + the
        # still-idle scalar engine) in parallel — dma_start issue time is the
        # startup bottleneck, ~300ns each, serialized per queue
        for c in range(CCH):
            rows = slice(c * 128, (c + 1) * 128)
            nc.scalar.dma_start(xt[c][:, 0:512], xT_d[rows, 0:512])
            nc.scalar.dma_start(wqt[c], wq_d[rows, :])
            nc.sync.dma_start(wkt[c], wk_d[rows, :])
            nc.sync.dma_start(wvt[c], wv_d[rows, :])
        for t5 in range(1, TT5):
            cols = slice(t5 * 512, (t5 + 1) * 512)
            for c in range(CCH):
                nc.sync.dma_start(xt[c][:, cols], xT_d[c * 128:(c + 1) * 128, cols])
        for p in range(NPAIR):
            nc.sync.dma_start(wpt[p], wp_d[p * 128:(p + 1) * 128, :])

        # ---- persistent activations ----
        qt = bigpool.tile([128, NPAIR, T], BF16)
        kt = bigpool.tile([128, NPAIR, T], BF16)
        # v_aug stationary is [128]: col 0 = ones (so the AV denominator lands
        # on PSUM partition 0 — reciprocal_approx_fast ignores input partition
        # offsets), cols 1-63 = zeros, cols 64-127 = the head's 64 channels
        # (64-aligned for the PSUM-partition-offset rules on the DVE multiply).
        vt = bigpool.tile([128, TT1, HPC, 128], BF16)
        yt = bigpool.tile([128, NPAIR, T], BF16)

        nc.vector.memset(vt[:, :, :, 0:64], 0.0)
        nc.vector.memset(vt[:, :, :, 0:1], 1.0)

        # ---- qkv stage emitters (stage j = qt/kt cols 512j.., v tt 4j..) --
        def emit_q_stage(j, w_sb, dst, bias):
            cols = slice(j * 512, (j + 1) * 512)
            for p in range(NPAIR):
                acc = ps.tile([128, 512], F32, tag="po", bufs=2,
                              name=f"qk{j}_{p}")
                for c in range(CCH):
                    mm(acc, w_sb[c][:, p * 128:(p + 1) * 128], xt[c][:, cols],
                       start=(c == 0), stop=(c == CCH - 1))
                nc.vector.tensor_scalar_add(dst[:, p, cols], acc,
                                            bias[:, p:p + 1])

        def emit_v_stage(j, half):
            # half 0: tt 4j,4j+1; half 1: tt 4j+2,4j+3
            acc = ps.tile([128, 2, HPC, HD], F32, tag="po", bufs=2,
                          name=f"v{j}_{half}")
            for i in range(2):
                tt = 4 * j + 2 * half + i
                for c in range(CCH):
                    mm(acc[:, i], xt[c][:, tt * 128:(tt + 1) * 128], wvt[c],
                       start=(c == 0), stop=(c == CCH - 1))
            for i in range(2):
                tt = 4 * j + 2 * half + i
                nc.vector.tensor_copy(vt[:, tt, :, 64:128], acc[:, i])

        def emit_stage(j):
            emit_q_stage(j, wqt, qt, bqs)
            emit_q_stage(j, wkt, kt, bks)
            emit_v_stage(j, 0)
            emit_v_stage(j, 1)

        # dummy matmuls while the first DMAs land: the PE clock needs ~3us of
        # continuous work to ramp 0.65 -> 2.4 GHz, so burn the DMA wait
        # ramping instead of hitting the first real matmuls cold
        for w in range(12):
            wp_ps = ps.tile([HD, 512], F32, tag="po", bufs=2,
                            name=f"warm_{w}")
            mm(wp_ps, ones64, warm, start=True, stop=True)

        emit_stage(0)

        # ---- attention + interleaved next-stage qkv + proj ----
        # Deferred-PE-work FIFO: AV matmuls, norm broadcasts and proj units are
        # queued and drained 1-2 iterations later so every PE instruction's
        # dependencies (exp on ScalarE, reciprocal on DVE) are satisfied by
        # the time the in-order engine queue reaches it.  FIFO order also
        # guarantees reads of a rotating PSUM buffer are emitted before the
        # next writer of that buffer (Tile tracks WAR by emission order).
        # entries: (tag, closure); tag = ("av", qst, p) | ("n2",) | ("proj",)
        pending = deque()

        def drain(keep, max_pop=None):
            pops = 0
            while len(pending) > keep:
                pending.popleft()[1]()
                pops += 1
                if max_pop is not None and pops >= max_pop:
                    break

        def drain_chain_avs(qst, p):
            # emit everything up to and including this chain's last AV so the
            # reciprocal's read of the accumulator is ordered after the writes
            while any(t[0] == ("av", qst, p) for t in pending):
                pending.popleft()[1]()

        def make_norm2(qst, p, hs, ytps_hs, sinv):
            def norm2():
                q0 = qst * 512
                rb = ps.tile([HD, 512], F32, tag="po", bufs=2,
                             name=f"rb_{qst}_{p}_{hs}")
                mm(rb, ones64, sinv, start=True, stop=True)
                rbs = wkpool.tile([HD, 512], FP16, tag="rbs", bufs=2,
                                  name=f"rbs_{qst}_{p}_{hs}")
                nc.vector.tensor_copy(rbs, rb)
                nc.vector.tensor_mul(
                    yt[64 * hs:64 * hs + 64, p, q0:q0 + 512],
                    ytps_hs[64:128, :], rbs)
            return norm2

        def make_proj(tt, nh):
            def proj():
                pso = ps.tile([128, 512], F32, tag="po", bufs=2,
                              name=f"pso_{tt}_{nh}")
                for p in range(NPAIR):
                    mm(pso,
                       yt[:, p, tt * 128:(tt + 1) * 128],
                       wpt[p][:, nh * 512:(nh + 1) * 512],
                       start=(p == 0), stop=(p == NPAIR - 1))
                osb = opool.tile([128, 512], BF16, tag="osb", bufs=3,
                                 name=f"osb_{tt}_{nh}")
                # the final q-supertile's copies run after the exp stream is
                # done — alternate them onto the idle scalar engine
                if tt >= 12 and nh == 1:
                    nc.scalar.copy(osb, pso)
                else:
                    nc.vector.tensor_copy(osb, pso)
                nc.sync.dma_start(
                    yp_d[tt * 128:(tt + 1) * 128,
                         nh * 512:(nh + 1) * 512], osb)
            return proj

        for qst in range(TT5):
            q0 = qst * 512
            nkb = 4 * qst + 4
            for p in range(NPAIR):
                ytps = [
                    ps.tile([128, 512], F32, tag="acc", bufs=2,
                            name=f"ytps_{qst}_{p}_{hs}")
                    for hs in range(2)
                ]
                # iterate the full-width diagonal block first (the psum
                # accumulation start must cover [0:512]), then off-diagonal
                # blocks, then the shrinking diagonal blocks — so the chain
                # ENDS on tiny exp calls and the chain-end serialization
                # (last exp -> last AV -> reciprocal) is short
                kb_order = ([4 * qst] + list(range(0, 4 * qst)) +
                            [4 * qst + 1, 4 * qst + 2, 4 * qst + 3])
                for kb_i, kb in enumerate(kb_order):
                    # interleave next qkv stage mid-loop to keep PE fed
                    if qst < TT5 - 1:
                        if p == 0 and kb_i == nkb // 2:
                            emit_q_stage(qst + 1, wqt, qt, bqs)
                        elif p == 1 and kb_i == 1:
                            emit_q_stage(qst + 1, wkt, kt, bks)
                        elif p == 1 and kb_i == nkb // 2:
                            emit_v_stage(qst + 1, 0)
                        elif p == 1 and kb_i == nkb - 1:
                            emit_v_stage(qst + 1, 1)
                    j = kb - 4 * qst
                    vlo = 128 * j if j >= 0 else 0
                    st = ps.tile([128, 2, 512], F32, tag="st", bufs=2,
                                 name=f"st_{qst}_{p}_{kb}")
                    for hs in range(2):
                        r = slice(64 * hs, 64 * hs + 64)
                        mm(st[:, hs, vlo:512],
                           kt[r, p, kb * 128:(kb + 1) * 128],
                           qt[r, p, q0 + vlo:q0 + 512],
                           tile_position=(64 * hs, 0),
                           start=True, stop=True)
                    ex = wkpool.tile([128, 2, 512], BF16, tag="ex", bufs=14,
                                     name=f"ex_{qst}_{p}_{kb}")
                    nc.scalar.activation(ex[:, :, vlo:512], st[:, :, vlo:512],
                                         EXP)
                    if j >= 0:
                        nc.vector.tensor_mul(ex[:, :, vlo:vlo + 128],
                                             ex[:, :, vlo:vlo + 128], mtri2)

                    def make_av(ytps, p, kb, vlo, ex, first, last):
                        def av():
                            for hs in range(2):
                                mm(ytps[hs][:, vlo:512],
                                   vt[:, kb, 2 * p + hs, :],
                                   ex[:, hs, vlo:512],
                                   start=first, stop=last)
                        return av
                    pending.append((("av", qst, p),
                                    make_av(ytps, p, kb, vlo, ex,
                                            kb_i == 0, kb_i == nkb - 1)))
                    drain(keep=6, max_pop=1)

                # end of this pair's kb loop: emit this chain's AVs, then the
                # reciprocal (reads the den row at PSUM partition 0)
                drain_chain_avs(qst, p)
                for hs in range(2):
                    sinv = wkpool.tile([1, 512], BF16, tag="sinv", bufs=2,
                                       name=f"sinv_{qst}_{p}_{hs}")
                    _c = RECIP_APPROX_FAST_CONSTS
                    nc.vector._custom_dve(
                        RECIPROCAL_APPROX_FAST, out=sinv,
                        in0=ytps[hs][0:1, :],
                        s0=_c["s0"], s1=_c["s1"], imm2=_c["imm2"])
                    pending.append((("n2",),
                                    make_norm2(qst, p, hs, ytps[hs], sinv)))

            for tt in range(4 * qst, 4 * qst + 4):
                for nh in range(2):
                    pending.append((("proj",), make_proj(tt, nh)))

        drain(keep=0)

        if dbg_d is not None:
            for nm, sb in (("qtd", qt), ("ktd", kt), ("vtd", vt),
                           ("ytd", yt)):
                if nm in dbg_d:
                    nc.sync.dma_start(dbg_d[nm], sb)


_CACHE = {}


def _get_nc():
    if "nc" not in _CACHE:
        _CACHE["nc"] = _build_program()
    return _CACHE["nc"]


def make_in_maps(x, w_attn, b_attn):
    """Shard the full inputs into 8 per-core input maps."""
    x = np.asarray(x, dtype=np.float32)
    w_attn = np.asarray(w_attn, dtype=np.float32)
    b_attn = np.asarray(b_attn, dtype=np.float32)
    mtri = (np.arange(128)[None, :] >= np.arange(128)[:, None]).astype(NPBF)
    in_maps = []
    for core in range(NCORES):
        b, g = divmod(core, 4)
        cs = slice(g * CL, (g + 1) * CL)
        ks = slice(C + g * CL, C + (g + 1) * CL)
        in_maps.append({
            "xT": np.ascontiguousarray(x[b].T).astype(NPBF),
            "wq": np.ascontiguousarray(SCALE * w_attn[:, cs]).astype(NPBF),
            "wk": np.ascontiguousarray(w_attn[:, ks]).astype(NPBF),
            "wv": None,  # filled below
            "wp": None,  # filled by caller (needs w_proj)
            "bqs": np.ascontiguousarray(
                (SCALE * b_attn[cs]).reshape(NPAIR, 128).T),
            "bks": np.ascontiguousarray(b_attn[ks].reshape(NPAIR, 128).T),
            "mtri": mtri,
        })
        vs = slice(2 * C + g * CL, 2 * C + (g + 1) * CL)
        in_maps[-1]["wv"] = np.ascontiguousarray(w_attn[:, vs]).astype(NPBF)
    return in_maps


def kernel(x, w_attn, b_attn, w_proj, b_proj, _trace=False):
    w_attn = np.asarray(w_attn, dtype=np.float32)
    b_attn = np.asarray(b_attn, dtype=np.float32)
    w_proj = np.asarray(w_proj, dtype=np.float32)
    b_proj = np.asarray(b_proj, dtype=np.float32)
    in_maps = make_in_maps(x, w_attn, b_attn)
    for core in range(NCORES):
        g = core % 4
        in_maps[core]["wp"] = np.ascontiguousarray(
            w_proj[g * CL:(g + 1) * CL, :]).astype(NPBF)
    nc = _get_nc()
    res = run_bass_kernel_spmd(nc, in_maps, core_ids=list(range(NCORES)),
                               trace=_trace)
    out = np.zeros((B, T, C), dtype=np.float32)
    for core in range(NCORES):
        out[core // 4] += np.asarray(res.results[core]["yp"],
                                     dtype=np.float32)
    # v-bias folds through the softmax (weights sum to 1) into the proj bias
    out += b_proj + b_attn[2 * C:] @ w_proj
    if _trace:
        kernel.last_result = res
    return out


# revision 42
# speedup vs baseline: 1.0122x; 1.0122x over previous
"""Causal self-attention Trainium2 kernel (B=2, T=2048, C=1024, H=16).

Sharding: 8 cores = 2 batches x 4 head-groups (4 heads/core, Megatron-style
column-parallel QKV + row-parallel proj; the row-parallel all-reduce is the
host-side partial sum in `kernel`).

Per-core strategy (bf16 matmul operands, fp32 PSUM accumulation):
  - qT/kT kept transposed [head_dim, T] with 2 heads packed per 128
    partitions; scores are computed transposed (sT[k, q] = k @ qT) with the
    two heads row-packed on the PE array (K=64 each, concurrent sub-arrays).
  - QKV projection work is software-pipelined into the attention loop:
    stage j (qt/kt cols 512j:512j+512, v rows) is emitted inside the
    attention of q-supertile j-1, so the scalar engine's exp stream runs
    concurrently with QKV matmuls instead of after them.
  - softmax in [k, q] layout: exp on ScalarE from PSUM (scores are O(1), no
    max subtraction); causality via restricted column ranges + one
    [128,128] triangular mask multiply per diagonal block.
  - v natural [T, 64] per head with a ones column, so AV emits softmax
    denominators as row 64 of PSUM for free.  Normalization: DVE
    reciprocal_approx_fast on the den row, f32r rank-1 broadcast matmul
    (1 cyc/row), fp16 broadcast tile, one DVE multiply.
  - v bias is folded into b_proj on the host (softmax weights sum to 1).
  - proj consumes yT directly; per-core partial [T, C] written as bf16 and
    summed on the host.
"""

import sys
from collections import deque

for _p in ("/opt/trn_rl_repo",):
    if _p not in sys.path:
        sys.path.insert(0, _p)

import ml_dtypes
import numpy as np

import concourse.bacc as bacc
import concourse.mybir as mybir
import concourse.tile as tile
from concourse.alu_op_type import AluOpType
from concourse.bass_utils import run_bass_kernel_spmd
from concourse.dve_ops import RECIP_APPROX_FAST_CONSTS, RECIPROCAL_APPROX_FAST

F32 = mybir.dt.float32
F32R = mybir.dt.float32r
BF16 = mybir.dt.bfloat16
FP16 = mybir.dt.float16
NPBF = ml_dtypes.bfloat16
EXP = mybir.ActivationFunctionType.Exp

B, T, C = 2, 2048, 1024
H, HD = 16, 64
HPC = 4          # heads per core
NPAIR = 2        # head pairs per core
CL = HPC * HD    # 256 local channels
NCORES = 8
SCALE = 0.125    # 1/sqrt(64), folded into wq/bqs on the host

TT5 = T // 512   # 4  q supertiles / qkv stages
TT1 = T // 128   # 16 t tiles / k blocks
CCH = C // 128   # 8  contraction chunks


def _build_program():
    nc = bacc.Bacc("TRN2", target_bir_lowering=False, debug=False)

    xT_d = nc.dram_tensor("xT", [C, T], BF16, kind="ExternalInput").ap()
    wq_d = nc.dram_tensor("wq", [C, CL], BF16, kind="ExternalInput").ap()
    wk_d = nc.dram_tensor("wk", [C, CL], BF16, kind="ExternalInput").ap()
    wv_d = nc.dram_tensor("wv", [C, CL], BF16, kind="ExternalInput").ap()
    wp_d = nc.dram_tensor("wp", [CL, C], BF16, kind="ExternalInput").ap()
    bqs_d = nc.dram_tensor("bqs", [128, NPAIR], F32, kind="ExternalInput").ap()
    bks_d = nc.dram_tensor("bks", [128, NPAIR], F32, kind="ExternalInput").ap()
    mtri_d = nc.dram_tensor("mtri", [128, 128], BF16, kind="ExternalInput").ap()
    yp_d = nc.dram_tensor("yp", [T, C], BF16, kind="ExternalOutput").ap()

    with tile.TileContext(nc) as tc:
        _attn_kernel(tc, xT_d, wq_d, wk_d, wv_d, wp_d, bqs_d, bks_d, mtri_d,
                     yp_d)
    nc.compile()
    return nc


def _attn_kernel(tc, xT_d, wq_d, wk_d, wv_d, wp_d, bqs_d, bks_d, mtri_d,
                 yp_d, dbg_d=None):
    nc = tc.nc
    mm = nc.tensor.matmul

    with (
        tc.tile_pool(name="const", bufs=1) as cpool,
        tc.tile_pool(name="big", bufs=1) as bigpool,
        tc.tile_pool(name="work", bufs=1) as wkpool,
        tc.tile_pool(name="outp", bufs=1) as opool,
        tc.tile_pool(name="ps", bufs=1, space="PSUM") as ps,
    ):
        # ---- constants ----
        bqs = cpool.tile([128, NPAIR], F32)
        nc.sync.dma_start(bqs, bqs_d)
        bks = cpool.tile([128, NPAIR], F32)
        nc.sync.dma_start(bks, bks_d)
        mtri2 = cpool.tile([128, 2, 128], BF16)
        nc.sync.dma_start(mtri2[:, 0, :], mtri_d)
        nc.sync.dma_start(mtri2[:, 1, :], mtri_d)
        ones64 = cpool.tile([1, HD], BF16)
        nc.vector.memset(ones64, 1.0)
        warm = cpool.tile([1, 512], BF16)
        nc.vector.memset(warm, 1.0)

        # ---- inputs: per-chunk tiles, DMAs issued in priority order ----
        xt = [bigpool.tile([128, T], BF16, name=f"xt{c}") for c in range(CCH)]
        wqt = [bigpool.tile([128, CL], BF16, name=f"wqt{c}") for c in range(CCH)]
        wkt = [bigpool.tile([128, CL], BF16, name=f"wkt{c}") for c in range(CCH)]
        wvt = [bigpool.tile([128, CL], BF16, name=f"wvt{c}") for c in range(CCH)]
        wpt = [bigpool.tile([128, C], BF16, name=f"wpt{p}") for p in range(NPAIR)]

        # stage-0 inputs first; issue from both HWDGE queues (SP + the
        # still-idle scalar engine) in parallel — dma_start issue time is the
        # startup bottleneck, ~300ns each, serialized per queue
        for c in range(CCH):
            rows = slice(c * 128, (c + 1) * 128)
            nc.scalar.dma_start(xt[c][:, 0:512], xT_d[rows, 0:512])
            nc.scalar.dma_start(wqt[c], wq_d[rows, :])
            nc.sync.dma_start(wkt[c], wk_d[rows, :])
            nc.sync.dma_start(wvt[c], wv_d[rows, :])
        for t5 in range(1, TT5):
            cols = slice(t5 * 512, (t5 + 1) * 512)
            for c in range(CCH):
                nc.sync.dma_start(xt[c][:, cols], xT_d[c * 128:(c + 1) * 128, cols])
        for p in range(NPAIR):
            nc.sync.dma_start(wpt[p], wp_d[p * 128:(p + 1) * 128, :])

        # ---- persistent activations ----
        qt = bigpool.tile([128, NPAIR, T], BF16)
        kt = bigpool.tile([128, NPAIR, T], BF16)
        # v_aug stationary is [128]: col 0 = ones (so the AV denominator lands
        # on PSUM partition 0 — reciprocal_approx_fast ignores input partition
        # offsets), cols 1-63 = zeros, cols 64-127 = the head's 64 channels
        # (64-aligned for the PSUM-partition-offset rules on the DVE multiply).
        vt = bigpool.tile([128, TT1, HPC, 128], BF16)
        yt = bigpool.tile([128, NPAIR, T], BF16)

        nc.vector.memset(vt[:, :, :, 0:64], 0.0)
        nc.vector.memset(vt[:, :, :, 0:1], 1.0)

        # ---- qkv stage emitters (stage j = qt/kt cols 512j.., v tt 4j..) --
        def emit_q_stage(j, w_sb, dst, bias):
            cols = slice(j * 512, (j + 1) * 512)
            for p in range(NPAIR):
                acc = ps.tile([128, 512], F32, tag="po", bufs=2,
                              name=f"qk{j}_{p}")
                for c in range(CCH):
                    mm(acc, w_sb[c][:, p * 128:(p + 1) * 128], xt[c][:, cols],
                       start=(c == 0), stop=(c == CCH - 1))
                nc.vector.tensor_scalar_add(dst[:, p, cols], acc,
                                            bias[:, p:p + 1])

        def emit_v_stage(j, half):
            # half 0: tt 4j,4j+1; half 1: tt 4j+2,4j+3
            acc = ps.tile([128, 2, HPC, HD], F32, tag="po", bufs=2,
                          name=f"v{j}_{half}")
            for i in range(2):
                tt = 4 * j + 2 * half + i
                for c in range(CCH):
                    mm(acc[:, i], xt[c][:, tt * 128:(tt + 1) * 128], wvt[c],
                       start=(c == 0), stop=(c == CCH - 1))
            for i in range(2):
                tt = 4 * j + 2 * half + i
                nc.vector.tensor_copy(vt[:, tt, :, 64:128], acc[:, i])

        def emit_stage(j):
            emit_q_stage(j, wqt, qt, bqs)
            emit_q_stage(j, wkt, kt, bks)
            emit_v_stage(j, 0)
            emit_v_stage(j, 1)

        emit_stage(0)

        # ---- attention + interleaved next-stage qkv + proj ----
        # Deferred-PE-work FIFO: AV matmuls, norm broadcasts and proj units are
        # queued and drained 1-2 iterations later so every PE instruction's
        # dependencies (exp on ScalarE, reciprocal on DVE) are satisfied by
        # the time the in-order engine queue reaches it.  FIFO order also
        # guarantees reads of a rotating PSUM buffer are emitted before the
        # next writer of that buffer (Tile tracks WAR by emission order).
        # entries: (tag, closure); tag = ("av", qst, p) | ("n2",) | ("proj",)
        pending = deque()

        def drain(keep, max_pop=None):
            pops = 0
            while len(pending) > keep:
                pending.popleft()[1]()
                pops += 1
                if max_pop is not None and pops >= max_pop:
                    break

        def drain_chain_avs(qst, p):
            # emit everything up to and including this chain's last AV so the
            # reciprocal's read of the accumulator is ordered after the writes
            while any(t[0] == ("av", qst, p) for t in pending):
                pending.popleft()[1]()

        def make_norm2(qst, p, hs, ytps_hs, sinv):
            def norm2():
                q0 = qst * 512
                rb = ps.tile([HD, 512], F32, tag="po", bufs=2,
                             name=f"rb_{qst}_{p}_{hs}")
                mm(rb, ones64, sinv, start=True, stop=True)
                rbs = wkpool.tile([HD, 512], FP16, tag="rbs", bufs=2,
                                  name=f"rbs_{qst}_{p}_{hs}")
                nc.vector.tensor_copy(rbs, rb)
                nc.vector.tensor_mul(
                    yt[64 * hs:64 * hs + 64, p, q0:q0 + 512],
                    ytps_hs[64:128, :], rbs)
            return norm2

        def make_proj(tt, nh):
            def proj():
                pso = ps.tile([128, 512], F32, tag="po", bufs=2,
                              name=f"pso_{tt}_{nh}")
                for p in range(NPAIR):
                    mm(pso,
                       yt[:, p, tt * 128:(tt + 1) * 128],
                       wpt[p][:, nh * 512:(nh + 1) * 512],
                       start=(p == 0), stop=(p == NPAIR - 1))
                osb = opool.tile([128, 512], BF16, tag="osb", bufs=3,
                                 name=f"osb_{tt}_{nh}")
                # the final q-supertile's copies run after the exp stream is
                # done — alternate them onto the idle scalar engine
                if tt >= 12 and (tt + nh) % 2 == 1:
                    nc.scalar.copy(osb, pso)
                else:
                    nc.vector.tensor_copy(osb, pso)
                nc.sync.dma_start(
                    yp_d[tt * 128:(tt + 1) * 128,
                         nh * 512:(nh + 1) * 512], osb)
            return proj

        for qst in range(TT5):
            q0 = qst * 512
            nkb = 4 * qst + 4
            for p in range(NPAIR):
                ytps = [
                    ps.tile([128, 512], F32, tag="acc", bufs=2,
                            name=f"ytps_{qst}_{p}_{hs}")
                    for hs in range(2)
                ]
                # iterate the full-width diagonal block first (the psum
                # accumulation start must cover [0:512]), then off-diagonal
                # blocks, then the shrinking diagonal blocks — so the chain
                # ENDS on tiny exp calls and the chain-end serialization
                # (last exp -> last AV -> reciprocal) is short
                kb_order = ([4 * qst] + list(range(0, 4 * qst)) +
                            [4 * qst + 1, 4 * qst + 2, 4 * qst + 3])
                for kb_i, kb in enumerate(kb_order):
                    # interleave next qkv stage mid-loop to keep PE fed
                    if qst < TT5 - 1:
                        if p == 0 and kb_i == nkb // 2:
                            emit_q_stage(qst + 1, wqt, qt, bqs)
                        elif p == 1 and kb_i == 1:
                            emit_q_stage(qst + 1, wkt, kt, bks)
                        elif p == 1 and kb_i == nkb // 2:
                            emit_v_stage(qst + 1, 0)
                        elif p == 1 and kb_i == nkb - 1:
                            emit_v_stage(qst + 1, 1)
                    j = kb - 4 * qst
                    vlo = 128 * j if j >= 0 else 0
                    st = ps.tile([128, 2, 512], F32, tag="st", bufs=2,
                                 name=f"st_{qst}_{p}_{kb}")
                    for hs in range(2):
                        r = slice(64 * hs, 64 * hs + 64)
                        mm(st[:, hs, vlo:512],
                           kt[r, p, kb * 128:(kb + 1) * 128],
                           qt[r, p, q0 + vlo:q0 + 512],
                           tile_position=(64 * hs, 0),
                           start=True, stop=True)
                    ex = wkpool.tile([128, 2, 512], BF16, tag="ex", bufs=14,
                                     name=f"ex_{qst}_{p}_{kb}")
                    nc.scalar.activation(ex[:, :, vlo:512], st[:, :, vlo:512],
                                         EXP)
                    if j >= 0:
                        nc.vector.tensor_mul(ex[:, :, vlo:vlo + 128],
                                             ex[:, :, vlo:vlo + 128], mtri2)

                    def make_av(ytps, p, kb, vlo, ex, first, last):
                        def av():
                            for hs in range(2):
                                mm(ytps[hs][:, vlo:512],
                                   vt[:, kb, 2 * p + hs, :],
                                   ex[:, hs, vlo:512],
                                   start=first, stop=last)
                        return av
                    pending.append((("av", qst, p),
                                    make_av(ytps, p, kb, vlo, ex,
                                            kb_i == 0, kb_i == nkb - 1)))
                    drain(keep=6, max_pop=1)

                # end of this pair's kb loop: emit this chain's AVs, then the
                # reciprocal (reads the den row at PSUM partition 0)
                drain_chain_avs(qst, p)
                for hs in range(2):
                    sinv = wkpool.tile([1, 512], BF16, tag="sinv", bufs=2,
                                       name=f"sinv_{qst}_{p}_{hs}")
                    _c = RECIP_APPROX_FAST_CONSTS
                    nc.vector._custom_dve(
                        RECIPROCAL_APPROX_FAST, out=sinv,
                        in0=ytps[hs][0:1, :],
                        s0=_c["s0"], s1=_c["s1"], imm2=_c["imm2"])
                    pending.append((("n2",),
                                    make_norm2(qst, p, hs, ytps[hs], sinv)))

            for tt in range(4 * qst, 4 * qst + 4):
                for nh in range(2):
                    pending.append((("proj",), make_proj(tt, nh)))

        drain(keep=0)

        if dbg_d is not None:
            for nm, sb in (("qtd", qt), ("ktd", kt), ("vtd", vt),
                           ("ytd", yt)):
                if nm in dbg_d:
                    nc.sync.dma_start(dbg_d[nm], sb)


_CACHE = {}


def _get_nc():
    if "nc" not in _CACHE:
        _CACHE["nc"] = _build_program()
    return _CACHE["nc"]


def make_in_maps(x, w_attn, b_attn):
    """Shard the full inputs into 8 per-core input maps."""
    x = np.asarray(x, dtype=np.float32)
    w_attn = np.asarray(w_attn, dtype=np.float32)
    b_attn = np.asarray(b_attn, dtype=np.float32)
    mtri = (np.arange(128)[None, :] >= np.arange(128)[:, None]).astype(NPBF)
    in_maps = []
    for core in range(NCORES):
        b, g = divmod(core, 4)
        cs = slice(g * CL, (g + 1) * CL)
        ks = slice(C + g * CL, C + (g + 1) * CL)
        in_maps.append({
            "xT": np.ascontiguousarray(x[b].T).astype(NPBF),
            "wq": np.ascontiguousarray(SCALE * w_attn[:, cs]).astype(NPBF),
            "wk": np.ascontiguousarray(w_attn[:, ks]).astype(NPBF),
            "wv": None,  # filled below
            "wp": None,  # filled by caller (needs w_proj)
            "bqs": np.ascontiguousarray(
                (SCALE * b_attn[cs]).reshape(NPAIR, 128).T),
            "bks": np.ascontiguousarray(b_attn[ks].reshape(NPAIR, 128).T),
            "mtri": mtri,
        })
        vs = slice(2 * C + g * CL, 2 * C + (g + 1) * CL)
        in_maps[-1]["wv"] = np.ascontiguousarray(w_attn[:, vs]).astype(NPBF)
    return in_maps


def kernel(x, w_attn, b_attn, w_proj, b_proj, _trace=False):
    w_attn = np.asarray(w_attn, dtype=np.float32)
    b_attn = np.asarray(b_attn, dtype=np.float32)
    w_proj = np.asarray(w_proj, dtype=np.float32)
    b_proj = np.asarray(b_proj, dtype=np.float32)
    in_maps = make_in_maps(x, w_attn, b_attn)
    for core in range(NCORES):
        g = core % 4
        in_maps[core]["wp"] = np.ascontiguousarray(
            w_proj[g * CL:(g + 1) * CL, :]).astype(NPBF)
    nc = _get_nc()
    res = run_bass_kernel_spmd(nc, in_maps, core_ids=list(range(NCORES)),
                               trace=_trace)
    out = np.zeros((B, T, C), dtype=np.float32)
    for core in range(NCORES):
        out[core // 4] += np.asarray(res.results[core]["yp"],
                                     dtype=np.float32)
    # v-bias folds through the softmax (weights sum to 1) into the proj bias
    out += b_proj + b_attn[2 * C:] @ w_proj
    if _trace:
        kernel.last_result = res
    return out


# revision 45
# speedup vs baseline: 1.0169x; 1.0047x over previous
"""Causal self-attention Trainium2 kernel (B=2, T=2048, C=1024, H=16).

Sharding: 8 cores = 2 batches x 4 head-groups (4 heads/core, Megatron-style
column-parallel QKV + row-parallel proj; the row-parallel all-reduce is the
host-side partial sum in `kernel`).

Per-core strategy (bf16 matmul operands, fp32 PSUM accumulation):
  - qT/kT kept transposed [head_dim, T] with 2 heads packed per 128
    partitions; scores are computed transposed (sT[k, q] = k @ qT) with the
    two heads row-packed on the PE array (K=64 each, concurrent sub-arrays).
  - QKV projection work is software-pipelined into the attention loop:
    stage j (qt/kt cols 512j:512j+512, v rows) is emitted inside the
    attention of q-supertile j-1, so the scalar engine's exp stream runs
    concurrently with QKV matmuls instead of after them.
  - softmax in [k, q] layout: exp on ScalarE from PSUM (scores are O(1), no
    max subtraction); causality via restricted column ranges + one
    [128,128] triangular mask multiply per diagonal block.
  - v natural [T, 64] per head with a ones column, so AV emits softmax
    denominators as row 64 of PSUM for free.  Normalization: DVE
    reciprocal_approx_fast on the den row, f32r rank-1 broadcast matmul
    (1 cyc/row), fp16 broadcast tile, one DVE multiply.
  - v bias is folded into b_proj on the host (softmax weights sum to 1).
  - proj consumes yT directly; per-core partial [T, C] written as bf16 and
    summed on the host.
"""

import sys
from collections import deque

for _p in ("/opt/trn_rl_repo",):
    if _p not in sys.path:
        sys.path.insert(0, _p)

import ml_dtypes
import numpy as np

import concourse.bacc as bacc
import concourse.mybir as mybir
import concourse.tile as tile
from concourse.alu_op_type import AluOpType
from concourse.bass_utils import run_bass_kernel_spmd
from concourse.dve_ops import RECIP_APPROX_FAST_CONSTS, RECIPROCAL_APPROX_FAST

F32 = mybir.dt.float32
F32R = mybir.dt.float32r
BF16 = mybir.dt.bfloat16
FP16 = mybir.dt.float16
NPBF = ml_dtypes.bfloat16
EXP = mybir.ActivationFunctionType.Exp

B, T, C = 2, 2048, 1024
H, HD = 16, 64
HPC = 4          # heads per core
NPAIR = 2        # head pairs per core
CL = HPC * HD    # 256 local channels
NCORES = 8
SCALE = 0.125    # 1/sqrt(64), folded into wq/bqs on the host

TT5 = T // 512   # 4  q supertiles / qkv stages
TT1 = T // 128   # 16 t tiles / k blocks
CCH = C // 128   # 8  contraction chunks


def _build_program():
    nc = bacc.Bacc("TRN2", target_bir_lowering=False, debug=False)

    xT_d = nc.dram_tensor("xT", [C, T], BF16, kind="ExternalInput").ap()
    wq_d = nc.dram_tensor("wq", [C, CL], BF16, kind="ExternalInput").ap()
    wk_d = nc.dram_tensor("wk", [C, CL], BF16, kind="ExternalInput").ap()
    wv_d = nc.dram_tensor("wv", [C, CL], BF16, kind="ExternalInput").ap()
    wp_d = nc.dram_tensor("wp", [CL, C], BF16, kind="ExternalInput").ap()
    bqs_d = nc.dram_tensor("bqs", [128, NPAIR], F32, kind="ExternalInput").ap()
    bks_d = nc.dram_tensor("bks", [128, NPAIR], F32, kind="ExternalInput").ap()
    mtri_d = nc.dram_tensor("mtri", [128, 128], BF16, kind="ExternalInput").ap()
    yp_d = nc.dram_tensor("yp", [T, C], BF16, kind="ExternalOutput").ap()

    with tile.TileContext(nc) as tc:
        _attn_kernel(tc, xT_d, wq_d, wk_d, wv_d, wp_d, bqs_d, bks_d, mtri_d,
                     yp_d)
    nc.compile()
    return nc


def _attn_kernel(tc, xT_d, wq_d, wk_d, wv_d, wp_d, bqs_d, bks_d, mtri_d,
                 yp_d, dbg_d=None):
    nc = tc.nc
    mm = nc.tensor.matmul

    with (
        tc.tile_pool(name="const", bufs=1) as cpool,
        tc.tile_pool(name="big", bufs=1) as bigpool,
        tc.tile_pool(name="work", bufs=1) as wkpool,
        tc.tile_pool(name="outp", bufs=1) as opool,
        tc.tile_pool(name="ps", bufs=1, space="PSUM") as ps,
    ):
        # ---- constants ----
        bqs = cpool.tile([128, NPAIR], F32)
        nc.sync.dma_start(bqs, bqs_d)
        bks = cpool.tile([128, NPAIR], F32)
        nc.sync.dma_start(bks, bks_d)
        mtri2 = cpool.tile([128, 2, 128], BF16)
        nc.sync.dma_start(mtri2[:, 0, :], mtri_d)
        nc.sync.dma_start(mtri2[:, 1, :], mtri_d)
        ones64 = cpool.tile([1, HD], BF16)
        nc.vector.memset(ones64, 1.0)
        warm = cpool.tile([1, 512], BF16)
        nc.vector.memset(warm, 1.0)

        # ---- inputs: per-chunk tiles, DMAs issued in priority order ----
        xt = [bigpool.tile([128, T], BF16, name=f"xt{c}") for c in range(CCH)]
        wqt = [bigpool.tile([128, CL], BF16, name=f"wqt{c}") for c in range(CCH)]
        wkt = [bigpool.tile([128, CL], BF16, name=f"wkt{c}") for c in range(CCH)]
        wvt = [bigpool.tile([128, CL], BF16, name=f"wvt{c}") for c in range(CCH)]
        wpt = [bigpool.tile([128, C], BF16, name=f"wpt{p}") for p in range(NPAIR)]

        # stage-0 inputs first; issue from both HWDGE queues (SP + the
        # still-idle scalar engine) in parallel — dma_start issue time is the
        # startup bottleneck, ~300ns each, serialized per queue
        for c in range(CCH):
            rows = slice(c * 128, (c + 1) * 128)
            nc.scalar.dma_start(xt[c][:, 0:512], xT_d[rows, 0:512])
            nc.scalar.dma_start(wqt[c], wq_d[rows, :])
            nc.sync.dma_start(wkt[c], wk_d[rows, :])
            nc.sync.dma_start(wvt[c], wv_d[rows, :])
        for t5 in range(1, TT5):
            cols = slice(t5 * 512, (t5 + 1) * 512)
            for c in range(CCH):
                nc.sync.dma_start(xt[c][:, cols], xT_d[c * 128:(c + 1) * 128, cols])
        for p in range(NPAIR):
            nc.sync.dma_start(wpt[p], wp_d[p * 128:(p + 1) * 128, :])

        # ---- persistent activations ----
        qt = bigpool.tile([128, NPAIR, T], BF16)
        kt = bigpool.tile([128, NPAIR, T], BF16)
        # v_aug stationary is [128]: col 0 = ones (so the AV denominator lands
        # on PSUM partition 0 — reciprocal_approx_fast ignores input partition
        # offsets), cols 1-63 = zeros, cols 64-127 = the head's 64 channels
        # (64-aligned for the PSUM-partition-offset rules on the DVE multiply).
        vt = bigpool.tile([128, TT1, HPC, 128], BF16)
        yt = bigpool.tile([128, NPAIR, T], BF16)

        nc.vector.memset(vt[:, :, :, 0:64], 0.0)
        nc.vector.memset(vt[:, :, :, 0:1], 1.0)

        # ---- qkv stage emitters (stage j = qt/kt cols 512j.., v tt 4j..) --
        def emit_q_stage(j, w_sb, dst, bias):
            cols = slice(j * 512, (j + 1) * 512)
            for p in range(NPAIR):
                acc = ps.tile([128, 512], F32, tag="po", bufs=2,
                              name=f"qk{j}_{p}")
                for c in range(CCH):
                    mm(acc, w_sb[c][:, p * 128:(p + 1) * 128], xt[c][:, cols],
                       start=(c == 0), stop=(c == CCH - 1))
                nc.vector.tensor_scalar_add(dst[:, p, cols], acc,
                                            bias[:, p:p + 1])

        def emit_v_stage(j, half):
            # half 0: tt 4j,4j+1; half 1: tt 4j+2,4j+3
            acc = ps.tile([128, 2, HPC, HD], F32, tag="po", bufs=2,
                          name=f"v{j}_{half}")
            for i in range(2):
                tt = 4 * j + 2 * half + i
                for c in range(CCH):
                    mm(acc[:, i], xt[c][:, tt * 128:(tt + 1) * 128], wvt[c],
                       start=(c == 0), stop=(c == CCH - 1))
            for i in range(2):
                tt = 4 * j + 2 * half + i
                nc.vector.tensor_copy(vt[:, tt, :, 64:128], acc[:, i])

        def emit_stage(j):
            emit_q_stage(j, wqt, qt, bqs)
            emit_q_stage(j, wkt, kt, bks)
            emit_v_stage(j, 0)
            emit_v_stage(j, 1)

        emit_stage(0)

        # ---- attention + interleaved next-stage qkv + proj ----
        # Deferred-PE-work FIFO: AV matmuls, norm broadcasts and proj units are
        # queued and drained 1-2 iterations later so every PE instruction's
        # dependencies (exp on ScalarE, reciprocal on DVE) are satisfied by
        # the time the in-order engine queue reaches it.  FIFO order also
        # guarantees reads of a rotating PSUM buffer are emitted before the
        # next writer of that buffer (Tile tracks WAR by emission order).
        # entries: (tag, closure); tag = ("av", qst, p) | ("n2",) | ("proj",)
        pending = deque()

        def drain(keep, max_pop=None):
            pops = 0
            while len(pending) > keep:
                pending.popleft()[1]()
                pops += 1
                if max_pop is not None and pops >= max_pop:
                    break

        def drain_chain_avs(qst, p):
            # emit everything up to and including this chain's last AV so the
            # reciprocal's read of the accumulator is ordered after the writes
            while any(t[0] == ("av", qst, p) for t in pending):
                pending.popleft()[1]()

        def make_norm2(qst, p, hs, ytps_hs, sinv):
            def norm2():
                q0 = qst * 512
                rb = ps.tile([HD, 512], F32, tag="po", bufs=2,
                             name=f"rb_{qst}_{p}_{hs}")
                mm(rb, ones64, sinv, start=True, stop=True)
                rbs = wkpool.tile([HD, 512], FP16, tag="rbs", bufs=2,
                                  name=f"rbs_{qst}_{p}_{hs}")
                nc.vector.tensor_copy(rbs, rb)
                nc.vector.tensor_mul(
                    yt[64 * hs:64 * hs + 64, p, q0:q0 + 512],
                    ytps_hs[64:128, :], rbs)
            return norm2

        def make_proj(tt, nh):
            def proj():
                pso = ps.tile([128, 512], F32, tag="po", bufs=2,
                              name=f"pso_{tt}_{nh}")
                for p in range(NPAIR):
                    mm(pso,
                       yt[:, p, tt * 128:(tt + 1) * 128],
                       wpt[p][:, nh * 512:(nh + 1) * 512],
                       start=(p == 0), stop=(p == NPAIR - 1))
                osb = opool.tile([128, 512], BF16, tag="osb", bufs=3,
                                 name=f"osb_{tt}_{nh}")
                # the final q-supertile's copies run after the exp stream is
                # done — alternate them onto the idle scalar engine
                if tt >= 12 and nh == 1:
                    nc.scalar.copy(osb, pso)
                else:
                    nc.vector.tensor_copy(osb, pso)
                nc.sync.dma_start(
                    yp_d[tt * 128:(tt + 1) * 128,
                         nh * 512:(nh + 1) * 512], osb)
            return proj

        for qst in range(TT5):
            q0 = qst * 512
            nkb = 4 * qst + 4
            for p in range(NPAIR):
                ytps = [
                    ps.tile([128, 512], F32, tag="acc", bufs=2,
                            name=f"ytps_{qst}_{p}_{hs}")
                    for hs in range(2)
                ]
                # iterate the full-width diagonal block first (the psum
                # accumulation start must cover [0:512]), then off-diagonal
                # blocks, then the shrinking diagonal blocks — so the chain
                # ENDS on tiny exp calls and the chain-end serialization
                # (last exp -> last AV -> reciprocal) is short
                kb_order = ([4 * qst] + list(range(0, 4 * qst)) +
                            [4 * qst + 1, 4 * qst + 2, 4 * qst + 3])
                for kb_i, kb in enumerate(kb_order):
                    # interleave next qkv stage mid-loop to keep PE fed
                    if qst < TT5 - 1:
                        if p == 0 and kb_i == nkb // 2:
                            emit_q_stage(qst + 1, wqt, qt, bqs)
                        elif p == 1 and kb_i == 1:
                            emit_q_stage(qst + 1, wkt, kt, bks)
                        elif p == 1 and kb_i == nkb // 2:
                            emit_v_stage(qst + 1, 0)
                        elif p == 1 and kb_i == nkb - 1:
                            emit_v_stage(qst + 1, 1)
                    j = kb - 4 * qst
                    vlo = 128 * j if j >= 0 else 0
                    st = ps.tile([128, 2, 512], F32, tag="st", bufs=2,
                                 name=f"st_{qst}_{p}_{kb}")
                    for hs in range(2):
                        r = slice(64 * hs, 64 * hs + 64)
                        mm(st[:, hs, vlo:512],
                           kt[r, p, kb * 128:(kb + 1) * 128],
                           qt[r, p, q0 + vlo:q0 + 512],
                           tile_position=(64 * hs, 0),
                           start=True, stop=True)
                    ex = wkpool.tile([128, 2, 512], BF16, tag="ex", bufs=16,
                                     name=f"ex_{qst}_{p}_{kb}")
                    nc.scalar.activation(ex[:, :, vlo:512], st[:, :, vlo:512],
                                         EXP)
                    if j >= 0:
                        nc.vector.tensor_mul(ex[:, :, vlo:vlo + 128],
                                             ex[:, :, vlo:vlo + 128], mtri2)

                    def make_av(ytps, p, kb, vlo, ex, first, last):
                        def av():
                            for hs in range(2):
                                mm(ytps[hs][:, vlo:512],
                                   vt[:, kb, 2 * p + hs, :],
                                   ex[:, hs, vlo:512],
                                   start=first, stop=last)
                        return av
                    pending.append((("av", qst, p),
                                    make_av(ytps, p, kb, vlo, ex,
                                            kb_i == 0, kb_i == nkb - 1)))
                    drain(keep=8, max_pop=1)

                # end of this pair's kb loop: emit this chain's AVs, then the
                # reciprocal (reads the den row at PSUM partition 0)
                drain_chain_avs(qst, p)
                for hs in range(2):
                    sinv = wkpool.tile([1, 512], BF16, tag="sinv", bufs=2,
                                       name=f"sinv_{qst}_{p}_{hs}")
                    _c = RECIP_APPROX_FAST_CONSTS
                    nc.vector._custom_dve(
                        RECIPROCAL_APPROX_FAST, out=sinv,
                        in0=ytps[hs][0:1, :],
                        s0=_c["s0"], s1=_c["s1"], imm2=_c["imm2"])
                    pending.append((("n2",),
                                    make_norm2(qst, p, hs, ytps[hs], sinv)))

            for tt in range(4 * qst, 4 * qst + 4):
                for nh in range(2):
                    pending.append((("proj",), make_proj(tt, nh)))

        drain(keep=0)

        if dbg_d is not None:
            for nm, sb in (("qtd", qt), ("ktd", kt), ("vtd", vt),
                           ("ytd", yt)):
                if nm in dbg_d:
                    nc.sync.dma_start(dbg_d[nm], sb)


_CACHE = {}


def _get_nc():
    if "nc" not in _CACHE:
        _CACHE["nc"] = _build_program()
    return _CACHE["nc"]


def make_in_maps(x, w_attn, b_attn):
    """Shard the full inputs into 8 per-core input maps."""
    x = np.asarray(x, dtype=np.float32)
    w_attn = np.asarray(w_attn, dtype=np.float32)
    b_attn = np.asarray(b_attn, dtype=np.float32)
    mtri = (np.arange(128)[None, :] >= np.arange(128)[:, None]).astype(NPBF)
    in_maps = []
    for core in range(NCORES):
        b, g = divmod(core, 4)
        cs = slice(g * CL, (g + 1) * CL)
        ks = slice(C + g * CL, C + (g + 1) * CL)
        in_maps.append({
            "xT": np.ascontiguousarray(x[b].T).astype(NPBF),
            "wq": np.ascontiguousarray(SCALE * w_attn[:, cs]).astype(NPBF),
            "wk": np.ascontiguousarray(w_attn[:, ks]).astype(NPBF),
            "wv": None,  # filled below
            "wp": None,  # filled by caller (needs w_proj)
            "bqs": np.ascontiguousarray(
                (SCALE * b_attn[cs]).reshape(NPAIR, 128).T),
            "bks": np.ascontiguousarray(b_attn[ks].reshape(NPAIR, 128).T),
            "mtri": mtri,
        })
        vs = slice(2 * C + g * CL, 2 * C + (g + 1) * CL)
        in_maps[-1]["wv"] = np.ascontiguousarray(w_attn[:, vs]).astype(NPBF)
    return in_maps


def kernel(x, w_attn, b_attn, w_proj, b_proj, _trace=False):
    w_attn = np.asarray(w_attn, dtype=np.float32)
    b_attn = np.asarray(b_attn, dtype=np.float32)
    w_proj = np.asarray(w_proj, dtype=np.float32)
    b_proj = np.asarray(b_proj, dtype=np.float32)
    in_maps = make_in_maps(x, w_attn, b_attn)
    for core in range(NCORES):
        g = core % 4
        in_maps[core]["wp"] = np.ascontiguousarray(
            w_proj[g * CL:(g + 1) * CL, :]).astype(NPBF)
    nc = _get_nc()
    res = run_bass_kernel_spmd(nc, in_maps, core_ids=list(range(NCORES)),
                               trace=_trace)
    out = np.zeros((B, T, C), dtype=np.float32)
    for core in range(NCORES):
        out[core // 4] += np.asarray(res.results[core]["yp"],
                                     dtype=np.float32)
    # v-bias folds through the softmax (weights sum to 1) into the proj bias
    out += b_proj + b_attn[2 * C:] @ w_proj
    if _trace:
        kernel.last_result = res
    return out
